# revision 9
# baseline (speedup 1.0000x reference)
"""GPT forward on 8 Trainium2 NeuronCores (Bass/Tile), sequence-parallel.

2 groups of 4 cores; group g = batch sample g. Core c in its group owns
query tiles {c, 7-c} (2 x 128 tokens) -> causal attention work balanced
(9 key-tiles total per core). One SPMD program: causal structure is
data-driven via per-core multiplicative fp16 masks applied post-exp;
local q-tile j=0 attends a fixed 4-key-tile prefix, j=1 attends all 8
(extra tiles masked to 0).

v1 perf restructure vs baseline:
  - attention pipelined per head-pair: S (row-group paired across the two
    heads of a feature chunk), exp (ACT), multiplicative mask (DVE), O,
    with double-buffered PSUM pools so PE/ACT/DVE overlap across units.
  - PSUM: mm x2 (1 bank each), pS4 x2 (1 bank), pS8 x2 (2 banks) = 8.
  - fc2 accumulates kg-partials into x_sb via DVE adds (frees PSUM banks).
  - LN rsqrt via exp(-0.5*ln(var+eps)): keeps ACT on one table set (Exp/Ln)
    avoiding the ~2.7us Sqrt<->Exp table reloads per layer.
  - all big weights stream through one 4-deep pool of [P,EC,1024] fp16
    blocks, DMA'd from the Scalar-engine HWDGE ring (Sync ring stays free
    for activations/KV/collective bounces).
  - no keep_warm serial chain; small PE-only warm filler during AG wait.

Layouts per core:
  residual x token-major [128, 2, 1024] fp32 (partition = token%128)
  matmul operands fp16 (LN gammas folded into weights on host),
  PSUM fp32. Biases: per-partition activation biases (Q/K/fc1) or
  ones-row augment matmuls (V/Wo/fc2/lm-head). All exact.
Per layer: one AllGather of (K feature-major, V token-major) fp16 within
each 4-core group.
"""
import sys
from contextlib import ExitStack

sys.path.insert(0, "/opt/trn_rl_repo")
sys.path.insert(0, "/root/.axon_site")

import numpy as np


# -- inline NTFF-trace shim (best-effort; tracing is optional) --------------
def _install_ntff_shim():
    import types
    try:
        import antenv.axon_hooks  # noqa: F401  (already present)
        return
    except ImportError:
        pass
    try:
        mod = types.ModuleType("antenv.axon_hooks")
        _h = [None]
        mod.set_axon_ntff_profile_hook = lambda h: _h.__setitem__(0, h)
        mod.get_axon_ntff_profile_hook = lambda: _h[0]
        sys.modules["antenv.axon_hooks"] = mod
        from trn_agent_boot.trn_boot import _ntff_profile_via_ctypes
        h = _ntff_profile_via_ctypes("/opt/axon/libaxon_pjrt.so")
        if h is not None:
            mod.set_axon_ntff_profile_hook(h)
    except Exception:
        pass


_install_ntff_shim()

from concourse import bacc, mybir, tile
from concourse.bass import ts
from concourse.bass_utils import run_bass_kernel_spmd

P = 128
L, H, E, T, B, V = 8, 16, 1024, 1024, 2, 800
D = E // H            # 64
E4 = 4 * E
NT = T // P           # 8 true token tiles per sample
EC = E // P           # 8 feature chunks
HPC = P // D          # 2 heads per feature chunk
N0, N1 = NT // 2, NT  # key-tile counts for local q-tile 0 / 1
NKT = N0 + N1         # 12 mask slots
KSZ = EC * P * 2 * P  # fp16 elems of K block in kv buffer (= 262144)
VSZ = 2 * P * E

f32 = mybir.dt.float32
f16 = mybir.dt.float16
AF = mybir.ActivationFunctionType
ALU = mybir.AluOpType

REPLICA_GROUPS = [[0, 1, 2, 3], [4, 5, 6, 7]]


def core_qtiles(c):
    return [c, NT - 1 - c]


def tile_owner(tau):
    """(rank-in-group, slot) that computed true token tile tau."""
    return (tau, 0) if tau < NT // 2 else (NT - 1 - tau, 1)


# ---------------------------------------------------------------- host prep
def prep_host(inputs):
    f = lambda k: np.asarray(inputs[k], np.float32)
    idx = f("idx")
    tok_table, pos_W, pos_b = f("tok_table"), f("pos_W"), f("pos_b")
    Wq, Wk, Wv, Wo, bo = f("Wq"), f("Wk"), f("Wv"), f("Wo"), f("bo")
    W1, b1, W2, b2 = f("W1"), f("b1"), f("W2"), f("b2")
    g1, be1 = f("ln1_g"), f("ln1_b")
    g2, be2 = f("ln2_g"), f("ln2_b")
    gf, bef = f("lnf_g"), f("lnf_b")
    lm_W, lm_b = f("lm_W"), f("lm_b")

    ids = np.clip(np.round(idx[..., 2] * 100.0 - 300.0), 0, V - 1).astype(np.int64)
    x0 = tok_table[ids] + idx[..., :2] @ pos_W + pos_b  # [B,T,E] fp32

    Wq_f = g1[:, :, None] * Wq
    Wk_f = g1[:, :, None] * Wk
    Wv_f = g1[:, :, None] * Wv
    W1_f = g2[:, :, None] * W1
    lm_W_f = gf[:, None] * lm_W

    h16 = lambda a: np.ascontiguousarray(a.astype(np.float16))
    com = {
        "Wq16": h16(Wq_f), "Wk16": h16(Wk_f), "Wv16": h16(Wv_f), "Wo16": h16(Wo),
        "W116": h16(W1_f), "W216": h16(W2), "lmW16": h16(lm_W_f),
        "qkbias": np.ascontiguousarray(np.stack(
            [np.einsum("le,lef->lf", be1, Wq_f),
             np.einsum("le,lef->lf", be1, Wk_f)], axis=1).astype(np.float32)),
        "fc1bias": np.ascontiguousarray(
            (np.einsum("le,lef->lf", be2, W1_f) + b1).astype(np.float32)),
        "vrow16": h16(np.einsum("le,lef->lf", be1, Wv_f)),   # [L,E]
        "worow16": h16(bo),                                   # [L,E]
        "w2row16": h16(b2),                                   # [L,E]
        "lmrow16": h16(bef @ lm_W_f + lm_b),                  # [V]
        "ident16": np.eye(P, dtype=np.float16),
    }

    # multiplicative post-exp mask, key-major: mask[k,q] = 1 iff k_glob <= q_glob
    tri = (np.arange(P)[:, None] <= np.arange(P)[None, :]).astype(np.float16)
    in_maps = []
    for r in range(8):
        g, c = divmod(r, 4)
        tiles = core_qtiles(c)
        xs = np.concatenate([x0[g, t * P:(t + 1) * P] for t in tiles], axis=0)
        mk = np.zeros((P, NKT, P), np.float16)
        for j, (n, off) in enumerate([(N0, 0), (N1, N0)]):
            tq = tiles[j]
            for kt in range(n):
                if kt == tq:
                    mk[:, off + kt, :] = tri
                elif kt < tq:
                    mk[:, off + kt, :] = 1.0
        m = dict(com)
        m["x0"] = np.ascontiguousarray(xs.astype(np.float32))
        m["mask16"] = np.ascontiguousarray(mk)
        in_maps.append(m)
    return in_maps


def assemble_output(results):
    out = np.empty((B, T, V), np.float32)
    for r in range(8):
        g, c = divmod(r, 4)
        lg = results[r]["logits"]
        for j, t in enumerate(core_qtiles(c)):
            out[g, t * P:(t + 1) * P] = lg[j * P:(j + 1) * P]
    return out


# ---------------------------------------------------------------- device build
def build(num_layers=L, debug_taps=()):
    nc = bacc.Bacc("TRN2", target_bir_lowering=False, debug=False, num_devices=8)
    NL = num_layers

    def din(name, shape, dt):
        return nc.dram_tensor(name, list(shape), dt, kind="ExternalInput").ap()

    x0_d = din("x0", [2 * P, E], f32)
    Wq_d = din("Wq16", [L, E, E], f16)
    Wk_d = din("Wk16", [L, E, E], f16)
    Wv_d = din("Wv16", [L, E, E], f16)
    Wo_d = din("Wo16", [L, E, E], f16)
    W1_d = din("W116", [L, E, E4], f16)
    W2_d = din("W216", [L, E4, E], f16)
    lmW_d = din("lmW16", [E, V], f16)
    qkb_d = din("qkbias", [L, 2, E], f32)
    fc1b_d = din("fc1bias", [L, E4], f32)
    vrow_d = din("vrow16", [L, E], f16)
    worow_d = din("worow16", [L, E], f16)
    w2row_d = din("w2row16", [L, E], f16)
    lmrow_d = din("lmrow16", [V], f16)
    ident_d = din("ident16", [P, P], f16)
    mask_d = din("mask16", [P, NKT, P], f16)

    logits_d = nc.dram_tensor("logits", [2 * P, V], f32, kind="ExternalOutput").ap()
    taps = {}
    for tname, tshape in debug_taps:
        taps[tname] = nc.dram_tensor(tname, list(tshape), f32,
                                     kind="ExternalOutput").ap()

    with tile.TileContext(nc) as tc, ExitStack() as ctx:
        ec = ctx.enter_context
        sb = ec(tc.tile_pool(name="sb", bufs=1))
        h16p = ec(tc.tile_pool(name="h16p", bufs=2))
        hT16p = ec(tc.tile_pool(name="hT16p", bufs=2))
        qfmp = ec(tc.tile_pool(name="qfmp", bufs=2))
        kvlp = ec(tc.tile_pool(name="kvlp", bufs=2))
        kvallp = ec(tc.tile_pool(name="kvallp", bufs=1))
        attp = ec(tc.tile_pool(name="attp", bufs=2))
        midp = ec(tc.tile_pool(name="midp", bufs=1))
        p16p = ec(tc.tile_pool(name="p16p", bufs=6))
        wsp = ec(tc.tile_pool(name="wsp", bufs=4))
        rowp = ec(tc.tile_pool(name="rowp", bufs=2))
        stp = ec(tc.tile_pool(name="stp", bufs=4))
        ps_mm = ec(tc.tile_pool(name="ps_mm", bufs=2, space="PSUM"))
        ps_s = ec(tc.tile_pool(name="ps_s", bufs=2, space="PSUM"))
        dramp = ec(tc.tile_pool(name="dramp", bufs=2, space="DRAM"))

        # ---- persistent tiles
        x_sb = sb.tile([P, 2, E], f32)
        nc.sync.dma_start(x_sb[:], x0_d.rearrange("(j p) e -> p j e", p=P))
        ident = sb.tile([P, P], f16)
        nc.sync.dma_start(ident[:], ident_d[:])
        mask16 = sb.tile([P, NKT, P], f16)
        nc.sync.dma_start(mask16[:], mask_d[:])
        ones_row = sb.tile([1, P], f16)
        nc.vector.memset(ones_row[:], 1.0)
        eps_col = sb.tile([P, 1], f32)
        nc.vector.memset(eps_col[:], 1e-5)
        junk16 = sb.tile([P, 512], f16)
        nc.vector.memset(junk16[:], 0.5)

        def warm_fill(uname, iters=70):
            """PE-only junk matmuls to keep HAM awake across an AG wait."""
            jp = ps_mm.tile([P, 512], f32, name=f"jp_{uname}", tag="mm")
            for i in range(iters):
                nc.tensor.matmul(jp[:], ident[:], junk16[:],
                                 start=(i == 0), stop=(i == iters - 1))

        def layer_norm(j, out16, uname):
            st = stp.tile([P, 2, 6], f32, name=f"st_{uname}", tag="st")
            for half in range(2):
                nc.vector.bn_stats(st[:, half, :], x_sb[:, j, ts(half, 512)])
            mv = stp.tile([P, 2], f32, name=f"mv_{uname}", tag="mv")
            nc.vector.bn_aggr(mv[:], st[:])
            sd = stp.tile([P, 1], f32, name=f"sd_{uname}", tag="sd")
            nc.scalar.activation(sd[:], mv[:, 1:2], AF.Sqrt, bias=eps_col[:])
            rs = stp.tile([P, 1], f32, name=f"rs_{uname}", tag="rs")
            nc.vector.reciprocal(rs[:], sd[:])
            nc.vector.tensor_scalar(
                out16[:], x_sb[:, j, :], mv[:, 0:1], rs[:],
                ALU.subtract, ALU.mult)

        def transpose_to(hT, h, uname):
            """h [P,2,E] fp16 token-major -> hT [P, EC, 2P] fp16 feature-major."""
            for j in range(2):
                for c in range(EC):
                    pt = ps_mm.tile([P, P], f16, name=f"pt_{uname}_{j}_{c}",
                                    tag="mm")
                    nc.tensor.transpose(pt[:], h[:, j, ts(c, P)], ident[:])
                    nc.scalar.copy(hT[:, c, ts(j, P)], pt[:])

        def opt2_matmul(out16, wsb, rhsT, uname, bias=None, relu=False):
            """out16 [P, n_mt, 2P] fm <- W.T @ rhsT; wsb [P, EC, n_mt*P]."""
            n_mt = wsb.shape[2] // P
            for mt in range(n_mt):
                pm = ps_mm.tile([P, 512], f32, name=f"pm_{uname}_{mt}", tag="mm")
                for ko in range(EC):
                    nc.tensor.matmul(
                        pm[:, :2 * P], wsb[:, ko, ts(mt, P)], rhsT[:, ko, :],
                        start=(ko == 0), stop=(ko == EC - 1))
                if relu:
                    nc.scalar.activation(out16[:, mt, :], pm[:, :2 * P], AF.Relu,
                                         bias=bias[:, mt:mt + 1])
                elif bias is not None:
                    nc.vector.tensor_scalar(
                        out16[:, mt, :], pm[:, :2 * P], bias[:, mt:mt + 1], None,
                        ALU.add)
                else:
                    nc.scalar.copy(out16[:, mt, :], pm[:, :2 * P])

        def wload(src_ap, uname):
            w = wsp.tile([P, EC, src_ap.shape[1]], f16, name=f"w_{uname}",
                         tag="wst")
            nc.gpsimd.dma_start(w[:], src_ap.rearrange("(ko p) m -> p ko m", p=P))
            return w

        # ================================================================ layers
        for l in range(NL):
            wq = wload(Wq_d[l], f"q{l}")
            wk = wload(Wk_d[l], f"k{l}")
            wv = wload(Wv_d[l], f"v{l}")
            qkb = rowp.tile([P, 2, EC], f32, name=f"qkb_{l}", tag="qkb")
            nc.sync.dma_start(qkb[:], qkb_d[l].rearrange("q (mt p) -> p q mt", p=P))
            vrow = rowp.tile([1, E], f16, name=f"vrow_{l}", tag="vrow")
            nc.sync.dma_start(vrow[:], vrow_d[l, None, :])

            # ---- LN1 -> h1 fp16, h1T
            h1 = h16p.tile([P, 2, E], f16, name=f"h1_{l}", tag="h16")
            for j in range(2):
                layer_norm(j, h1[:, j, :], f"l1_{l}_{j}")
            h1T = hT16p.tile([P, EC, 2 * P], f16, name=f"h1T_{l}", tag="hT")
            transpose_to(h1T, h1, f"h1_{l}")

            # ---- K feature-major first -> kv_in K region, launch AG_K early
            k_fm = kvlp.tile([P, EC, 2 * P], f16, name=f"kfm_{l}", tag="kfm")
            opt2_matmul(k_fm, wk, h1T, f"k{l}", bias=qkb[:, 1])
            kv_ink = dramp.tile([KSZ], f16, name=f"kvink_{l}", tag="kvink")
            nc.sync.dma_start(
                kv_ink.rearrange("(c p t) -> p c t", c=EC, p=P), k_fm[:])
            kv_outk = dramp.tile([4, KSZ], f16, name=f"kvoutk_{l}", tag="kvoutk")
            nc.gpsimd.collective_compute(
                "AllGather", ALU.bypass, replica_groups=REPLICA_GROUPS,
                ins=[kv_ink.opt()], outs=[kv_outk.opt()])

            # ---- V token-major (opt1) + ones-row bias, then AG_V
            v_tok = kvlp.tile([P, 2, E], f16, name=f"vtok_{l}", tag="vtok")
            for j in range(2):
                for nh in range(2):
                    pv = ps_mm.tile([P, 512], f32, name=f"pv_{l}_{j}_{nh}",
                                    tag="mm")
                    for ko in range(EC):
                        nc.tensor.matmul(
                            pv[:], h1T[:, ko, ts(j, P)], wv[:, ko, ts(nh, 512)],
                            start=(ko == 0), stop=False)
                    nc.tensor.matmul(pv[:], ones_row[:, :P],
                                     vrow[:, ts(nh, 512)], start=False, stop=True)
                    nc.scalar.copy(v_tok[:, j, ts(nh, 512)], pv[:])
            kv_inv = dramp.tile([VSZ], f16, name=f"kvinv_{l}", tag="kvinv")
            nc.sync.dma_start(
                kv_inv.rearrange("(j p e) -> p j e", j=2, p=P), v_tok[:])
            kv_outv = dramp.tile([4, VSZ], f16, name=f"kvoutv_{l}", tag="kvoutv")
            nc.gpsimd.collective_compute(
                "AllGather", ALU.bypass, replica_groups=REPLICA_GROUPS,
                ins=[kv_inv.opt()], outs=[kv_outv.opt()])

            # ---- Q projection overlaps the collectives
            q_fm = qfmp.tile([P, EC, 2 * P], f16, name=f"qfm_{l}", tag="qfm")
            opt2_matmul(q_fm, wq, h1T, f"q{l}", bias=qkb[:, 0])
            warm_fill(f"kw_{l}")

            # ---- load gathered K (true order) and V (ones-augmented)
            k_all = kvallp.tile([P, EC, T], f16, name=f"kall_{l}", tag="kall")
            v_aug = kvallp.tile([P, NT, H, D + 1], f16, name=f"vaug_{l}",
                                tag="vaug")
            nc.vector.memset(v_aug[:, :, :, D:D + 1], 1.0)
            for tau in range(NT):
                r, slot = tile_owner(tau)
                kblk = kv_outk[r].rearrange("(c p t) -> p c t", c=EC, p=P)
                nc.sync.dma_start(k_all[:, :, ts(tau, P)], kblk[:, :, ts(slot, P)])
            for tau in range(NT):
                r, slot = tile_owner(tau)
                vblk = kv_outv[r].rearrange(
                    "(j p h d) -> p j h d", j=2, p=P, h=H)
                nc.sync.dma_start(v_aug[:, tau, :, :D], vblk[:, slot])

            # ---- attention, pipelined per head-pair (ro=0/64 share a chunk)
            attn = attp.tile([P, 2, E], f16, name=f"attn_{l}", tag="attn")
            for pr in range(H // 2):
                co = pr
                pS = {}
                # S for both heads of the pair back-to-back: the two heads'
                # lhsT base partitions (0 / 64) land on distinct PE row
                # groups so their matmuls overlap in the array.
                for hi in range(2):
                    h = 2 * pr + hi
                    ro = hi * D
                    pS[hi] = ps_s.tile(
                        [P, NKT, P], f32, name=f"pS_{l}_{h}", tag="pS")
                    for j, (n_kt, off) in enumerate([(N0, 0), (N1, N0)]):
                        for kt in range(n_kt):
                            nc.tensor.matmul(
                                pS[hi][:, off + kt, :],
                                k_all[ro:ro + D, co, ts(kt, P)],
                                q_fm[ro:ro + D, co, ts(j, P)],
                                start=True, stop=True)
                p16 = {}
                for hi in range(2):
                    h = 2 * pr + hi
                    p16[hi] = p16p.tile([P, NKT, P], f16, name=f"p16_{l}_{h}",
                                        tag="p16")
                    nc.scalar.activation(
                        p16[hi][:], pS[hi][:],
                        AF.Exp, scale=float(D) ** -0.5)
                    # multiplicative causal mask (exact 0/1 triangular);
                    # alternate DVE / GpSimd so the two engines split the work
                    meng = nc.vector if hi == 0 else nc.gpsimd
                    meng.tensor_tensor(
                        p16[hi][:], p16[hi][:], mask16[:], ALU.mult)
                for hi in range(2):
                    h = 2 * pr + hi
                    for j, (n_kt, off) in enumerate([(N0, 0), (N1, N0)]):
                        pO = ps_mm.tile([P, 512], f32, name=f"pO_{l}_{h}_{j}",
                                        tag="mm")
                        for kt in range(n_kt):
                            nc.tensor.matmul(
                                pO[:, :D + 1], p16[hi][:, off + kt, :],
                                v_aug[:, kt, h, :],
                                start=(kt == 0), stop=(kt == n_kt - 1))
                        rz = stp.tile([P, 1], f32, name=f"rz_{l}_{h}_{j}",
                                      tag="rz")
                        nc.vector.reciprocal(rz[:], pO[:, D:D + 1])
                        nc.vector.tensor_scalar(
                            attn[:, j, h * D:(h + 1) * D], pO[:, :D],
                            rz[:], None, ALU.mult)

            # ---- attnT, Wo (opt1: lhsT=attnT chunk, rhs=Wo) + residual
            attnT = hT16p.tile([P, EC, 2 * P], f16, name=f"attnT_{l}", tag="hT")
            transpose_to(attnT, attn, f"at_{l}")
            wo = wload(Wo_d[l], f"o{l}")
            worow = rowp.tile([1, E], f16, name=f"worow_{l}", tag="vrow")
            nc.sync.dma_start(worow[:], worow_d[l, None, :])
            for j in range(2):
                for nh in range(2):
                    po = ps_mm.tile([P, 512], f32, name=f"po_{l}_{j}_{nh}",
                                    tag="mm")
                    for ko in range(EC):
                        nc.tensor.matmul(
                            po[:], attnT[:, ko, ts(j, P)], wo[:, ko, ts(nh, 512)],
                            start=(ko == 0), stop=False)
                    nc.tensor.matmul(po[:], ones_row[:, :P],
                                     worow[:, ts(nh, 512)], start=False, stop=True)
                    nc.vector.tensor_add(x_sb[:, j, ts(nh, 512)],
                                         x_sb[:, j, ts(nh, 512)], po[:])

            # ---- LN2 -> h2, h2T
            h2 = h16p.tile([P, 2, E], f16, name=f"h2_{l}", tag="h16")
            for j in range(2):
                layer_norm(j, h2[:, j, :], f"l2_{l}_{j}")
            h2T = hT16p.tile([P, EC, 2 * P], f16, name=f"h2T_{l}", tag="hT")
            transpose_to(h2T, h2, f"h2_{l}")

            # ---- fc1 (opt2, W1 streamed in 1024-col blocks) -> mid fp16
            fc1b = rowp.tile([P, 4 * EC], f32, name=f"fc1b_{l}", tag="fc1b")
            nc.sync.dma_start(fc1b[:], fc1b_d[l].rearrange("(mt p) -> p mt", p=P))
            mid = midp.tile([P, 4 * EC, 2 * P], f16, name=f"mid_{l}", tag="mid")
            for blk in range(4):
                w1b = wload(W1_d[l, :, ts(blk, 1024)], f"1_{l}_{blk}")
                for mi in range(8):
                    mt = blk * 8 + mi
                    pm = ps_mm.tile([P, 512], f32, name=f"pf_{l}_{blk}_{mi}",
                                    tag="mm")
                    for ko in range(EC):
                        nc.tensor.matmul(
                            pm[:, :2 * P], w1b[:, ko, ts(mi, P)], h2T[:, ko, :],
                            start=(ko == 0), stop=(ko == EC - 1))
                    nc.scalar.activation(mid[:, mt, :], pm[:, :2 * P], AF.Relu,
                                         bias=fc1b[:, mt:mt + 1])

            # ---- fc2 (opt1, W2 streamed in 1024-row k-groups); kg partials
            # accumulate into x_sb via DVE adds (keeps PSUM to 2 mm banks).
            w2row = rowp.tile([1, E], f16, name=f"w2row_{l}", tag="vrow")
            nc.sync.dma_start(w2row[:], w2row_d[l, None, :])
            for kg in range(4):
                w2b = wload(W2_d[l, ts(kg, 1024), :], f"2_{l}_{kg}")
                for j in range(2):
                    for nh in range(2):
                        pfc = ps_mm.tile([P, 512], f32,
                                         name=f"pfc_{l}_{kg}_{j}_{nh}", tag="mm")
                        for ko in range(EC):
                            nc.tensor.matmul(
                                pfc[:], mid[:, kg * EC + ko, ts(j, P)],
                                w2b[:, ko, ts(nh, 512)],
                                start=(ko == 0), stop=(kg < 3 and ko == EC - 1))
                        if kg == 3:
                            nc.tensor.matmul(pfc[:], ones_row[:, :P],
                                             w2row[:, ts(nh, 512)], start=False,
                                             stop=True)
                        nc.vector.tensor_add(x_sb[:, j, ts(nh, 512)],
                                             x_sb[:, j, ts(nh, 512)], pfc[:])

            if f"xout{l}" in taps:
                nc.sync.dma_start(
                    taps[f"xout{l}"].rearrange("(j p) e -> p j e", p=P), x_sb[:])

        # ================================================================ head
        hf = h16p.tile([P, 2, E], f16, name="hf", tag="h16")
        for j in range(2):
            layer_norm(j, hf[:, j, :], f"lf_{j}")
        hfT = hT16p.tile([P, EC, 2 * P], f16, name="hfT", tag="hT")
        transpose_to(hfT, hf, "hf")
        lmw = wsp.tile([P, EC, V], f16, name="lmw", tag="wst")
        nc.scalar.dma_start(lmw[:], lmW_d.rearrange("(ko p) v -> p ko v", p=P))
        lmrow = rowp.tile([1, V], f16, name="lmrow", tag="lmrow")
        nc.sync.dma_start(lmrow[:], lmrow_d[None, :])
        out_sb = attp.tile([P, 2, V], f32, name="out_sb", tag="attn")
        for j in range(2):
            for nh in range(2):
                nv = V // 2
                pl = ps_mm.tile([P, 512], f32, name=f"pl_{j}_{nh}", tag="mm")
                for ko in range(EC):
                    nc.tensor.matmul(
                        pl[:, :nv], hfT[:, ko, ts(j, P)],
                        lmw[:, ko, ts(nh, nv)], start=(ko == 0), stop=False)
                nc.tensor.matmul(pl[:, :nv], ones_row[:, :P],
                                 lmrow[:, ts(nh, nv)], start=False, stop=True)
                nc.scalar.copy(out_sb[:, j, ts(nh, nv)], pl[:, :nv])
        nc.sync.dma_start(logits_d.rearrange("(j p) v -> p j v", p=P), out_sb[:])

    nc.compile()
    return nc


# ---------------------------------------------------------------- entry
_CACHED = {}


def run(inputs, num_layers=L, debug_taps=(), trace=False):
    key = (num_layers, tuple(t[0] for t in debug_taps))
    if key not in _CACHED:
        _CACHED[key] = build(num_layers, debug_taps)
    nc = _CACHED[key]
    in_maps = prep_host(inputs)
    return run_bass_kernel_spmd(nc, in_maps, core_ids=list(range(8)), trace=trace)


LAST_EXEC_NS = None


def kernel(**inputs):
    res = run(inputs, num_layers=L, trace=False)
    return assemble_output(res.results)


# revision 15
# speedup vs baseline: 1.0543x; 1.0543x over previous
"""GPT forward on 8 Trainium2 NeuronCores (Bass/Tile), sequence-parallel.

2 groups of 4 cores; group g = batch sample g. Core c in its group owns
query tiles {c, 7-c} (2 x 128 tokens) -> causal attention work balanced
(9 key-tiles total per core). One SPMD program: causal structure is
data-driven via per-core multiplicative fp16 masks applied post-exp;
local q-tile j=0 attends a fixed 4-key-tile prefix, j=1 attends all 8
(extra tiles masked to 0).

v1 perf restructure vs baseline:
  - attention pipelined per head-pair: S (row-group paired across the two
    heads of a feature chunk), exp (ACT), multiplicative mask (DVE), O,
    with double-buffered PSUM pools so PE/ACT/DVE overlap across units.
  - PSUM: mm x2 (1 bank each), pS4 x2 (1 bank), pS8 x2 (2 banks) = 8.
  - fc2 accumulates kg-partials into x_sb via DVE adds (frees PSUM banks).
  - LN rsqrt via exp(-0.5*ln(var+eps)): keeps ACT on one table set (Exp/Ln)
    avoiding the ~2.7us Sqrt<->Exp table reloads per layer.
  - all big weights stream through one 4-deep pool of [P,EC,1024] fp16
    blocks, DMA'd from the Scalar-engine HWDGE ring (Sync ring stays free
    for activations/KV/collective bounces).
  - no keep_warm serial chain; small PE-only warm filler during AG wait.

Layouts per core:
  residual x token-major [128, 2, 1024] fp32 (partition = token%128)
  matmul operands fp16 (LN gammas folded into weights on host),
  PSUM fp32. Biases: per-partition activation biases (Q/K/fc1) or
  ones-row augment matmuls (V/Wo/fc2/lm-head). All exact.
Per layer: one AllGather of (K feature-major, V token-major) fp16 within
each 4-core group.
"""
import sys
from contextlib import ExitStack

sys.path.insert(0, "/opt/trn_rl_repo")
sys.path.insert(0, "/root/.axon_site")

import numpy as np


# -- inline NTFF-trace shim (best-effort; tracing is optional) --------------
def _install_ntff_shim():
    import types
    try:
        import antenv.axon_hooks  # noqa: F401  (already present)
        return
    except ImportError:
        pass
    try:
        mod = types.ModuleType("antenv.axon_hooks")
        _h = [None]
        mod.set_axon_ntff_profile_hook = lambda h: _h.__setitem__(0, h)
        mod.get_axon_ntff_profile_hook = lambda: _h[0]
        sys.modules["antenv.axon_hooks"] = mod
        from trn_agent_boot.trn_boot import _ntff_profile_via_ctypes
        h = _ntff_profile_via_ctypes("/opt/axon/libaxon_pjrt.so")
        if h is not None:
            mod.set_axon_ntff_profile_hook(h)
    except Exception:
        pass


_install_ntff_shim()

from concourse import bacc, mybir, tile
from concourse.bass import ts
from concourse.bass_utils import run_bass_kernel_spmd

P = 128
L, H, E, T, B, V = 8, 16, 1024, 1024, 2, 800
D = E // H            # 64
E4 = 4 * E
NT = T // P           # 8 true token tiles per sample
EC = E // P           # 8 feature chunks
HPC = P // D          # 2 heads per feature chunk
N0, N1 = NT // 2, NT  # key-tile counts for local q-tile 0 / 1
NKT = N0 + N1         # 12 mask slots
KSZ = EC * P * 2 * P  # fp16 elems of K block in kv buffer (= 262144)
VSZ = 2 * P * E

f32 = mybir.dt.float32
f16 = mybir.dt.float16
AF = mybir.ActivationFunctionType
ALU = mybir.AluOpType

REPLICA_GROUPS = [[0, 1, 2, 3], [4, 5, 6, 7]]


def core_qtiles(c):
    return [c, NT - 1 - c]


def tile_owner(tau):
    """(rank-in-group, slot) that computed true token tile tau."""
    return (tau, 0) if tau < NT // 2 else (NT - 1 - tau, 1)


# ---------------------------------------------------------------- host prep
def prep_host(inputs):
    f = lambda k: np.asarray(inputs[k], np.float32)
    idx = f("idx")
    tok_table, pos_W, pos_b = f("tok_table"), f("pos_W"), f("pos_b")
    Wq, Wk, Wv, Wo, bo = f("Wq"), f("Wk"), f("Wv"), f("Wo"), f("bo")
    W1, b1, W2, b2 = f("W1"), f("b1"), f("W2"), f("b2")
    g1, be1 = f("ln1_g"), f("ln1_b")
    g2, be2 = f("ln2_g"), f("ln2_b")
    gf, bef = f("lnf_g"), f("lnf_b")
    lm_W, lm_b = f("lm_W"), f("lm_b")

    ids = np.clip(np.round(idx[..., 2] * 100.0 - 300.0), 0, V - 1).astype(np.int64)
    x0 = tok_table[ids] + idx[..., :2] @ pos_W + pos_b  # [B,T,E] fp32

    Wq_f = g1[:, :, None] * Wq
    Wk_f = g1[:, :, None] * Wk
    Wv_f = g1[:, :, None] * Wv
    W1_f = g2[:, :, None] * W1
    lm_W_f = gf[:, None] * lm_W

    h16 = lambda a: np.ascontiguousarray(a.astype(np.float16))
    com = {
        "Wq16": h16(Wq_f), "Wk16": h16(Wk_f), "Wv16": h16(Wv_f), "Wo16": h16(Wo),
        "W116": h16(W1_f), "W216": h16(W2), "lmW16": h16(lm_W_f),
        "qkbias": np.ascontiguousarray(np.stack(
            [np.einsum("le,lef->lf", be1, Wq_f),
             np.einsum("le,lef->lf", be1, Wk_f)], axis=1).astype(np.float32)),
        "fc1bias": np.ascontiguousarray(
            (np.einsum("le,lef->lf", be2, W1_f) + b1).astype(np.float32)),
        "vrow16": h16(np.einsum("le,lef->lf", be1, Wv_f)),   # [L,E]
        "worow16": h16(bo),                                   # [L,E]
        "w2row16": h16(b2),                                   # [L,E]
        "lmrow16": h16(bef @ lm_W_f + lm_b),                  # [V]
        "ident16": np.eye(P, dtype=np.float16),
    }

    # multiplicative post-exp mask, key-major: mask[k,q] = 1 iff k_glob <= q_glob
    tri = (np.arange(P)[:, None] <= np.arange(P)[None, :]).astype(np.float16)
    in_maps = []
    for r in range(8):
        g, c = divmod(r, 4)
        tiles = core_qtiles(c)
        xs = np.concatenate([x0[g, t * P:(t + 1) * P] for t in tiles], axis=0)
        mk = np.zeros((P, NKT, P), np.float16)
        for j, n in ((0, N0), (1, N1)):
            tq = tiles[j]
            for kt in range(n):
                slot = 2 * kt + j if kt < N0 else N0 + kt
                if kt == tq:
                    mk[:, slot, :] = tri
                elif kt < tq:
                    mk[:, slot, :] = 1.0
        m = dict(com)
        m["x0"] = np.ascontiguousarray(xs.astype(np.float32))
        m["mask16"] = np.ascontiguousarray(mk)
        in_maps.append(m)
    return in_maps


def assemble_output(results):
    out = np.empty((B, T, V), np.float32)
    for r in range(8):
        g, c = divmod(r, 4)
        lg = results[r]["logits"]
        for j, t in enumerate(core_qtiles(c)):
            out[g, t * P:(t + 1) * P] = lg[j * P:(j + 1) * P]
    return out


# ---------------------------------------------------------------- device build
def build(num_layers=L, debug_taps=()):
    nc = bacc.Bacc("TRN2", target_bir_lowering=False, debug=False, num_devices=8)
    NL = num_layers

    def din(name, shape, dt):
        return nc.dram_tensor(name, list(shape), dt, kind="ExternalInput").ap()

    x0_d = din("x0", [2 * P, E], f32)
    Wq_d = din("Wq16", [L, E, E], f16)
    Wk_d = din("Wk16", [L, E, E], f16)
    Wv_d = din("Wv16", [L, E, E], f16)
    Wo_d = din("Wo16", [L, E, E], f16)
    W1_d = din("W116", [L, E, E4], f16)
    W2_d = din("W216", [L, E4, E], f16)
    lmW_d = din("lmW16", [E, V], f16)
    qkb_d = din("qkbias", [L, 2, E], f32)
    fc1b_d = din("fc1bias", [L, E4], f32)
    vrow_d = din("vrow16", [L, E], f16)
    worow_d = din("worow16", [L, E], f16)
    w2row_d = din("w2row16", [L, E], f16)
    lmrow_d = din("lmrow16", [V], f16)
    ident_d = din("ident16", [P, P], f16)
    mask_d = din("mask16", [P, NKT, P], f16)

    logits_d = nc.dram_tensor("logits", [2 * P, V], f32, kind="ExternalOutput").ap()
    taps = {}
    for tname, tshape in debug_taps:
        taps[tname] = nc.dram_tensor(tname, list(tshape), f32,
                                     kind="ExternalOutput").ap()

    with tile.TileContext(nc) as tc, ExitStack() as ctx:
        ec = ctx.enter_context
        sb = ec(tc.tile_pool(name="sb", bufs=1))
        h16p = ec(tc.tile_pool(name="h16p", bufs=2))
        hT16p = ec(tc.tile_pool(name="hT16p", bufs=2))
        qfmp = ec(tc.tile_pool(name="qfmp", bufs=2))
        kvlp = ec(tc.tile_pool(name="kvlp", bufs=2))
        kvallp = ec(tc.tile_pool(name="kvallp", bufs=1))
        attp = ec(tc.tile_pool(name="attp", bufs=2))
        midp = ec(tc.tile_pool(name="midp", bufs=1))
        p16p = ec(tc.tile_pool(name="p16p", bufs=10))
        wsp = ec(tc.tile_pool(name="wsp", bufs=3))
        rowp = ec(tc.tile_pool(name="rowp", bufs=2))
        stp = ec(tc.tile_pool(name="stp", bufs=4))
        ps_mm = ec(tc.tile_pool(name="ps_mm", bufs=2, space="PSUM"))
        ps_s = ec(tc.tile_pool(name="ps_s", bufs=2, space="PSUM"))
        dramp = ec(tc.tile_pool(name="dramp", bufs=2, space="DRAM"))

        # ---- persistent tiles
        x_sb = sb.tile([P, 2, E], f32)
        nc.sync.dma_start(x_sb[:], x0_d.rearrange("(j p) e -> p j e", p=P))
        ident = sb.tile([P, P], f16)
        nc.sync.dma_start(ident[:], ident_d[:])
        mask16 = sb.tile([P, NKT, P], f16)
        nc.sync.dma_start(mask16[:], mask_d[:])
        ones_row = sb.tile([1, P], f16)
        nc.vector.memset(ones_row[:], 1.0)
        eps_col = sb.tile([P, 1], f32)
        nc.vector.memset(eps_col[:], 1e-5)
        junk16 = sb.tile([P, 512], f16)
        nc.vector.memset(junk16[:], 0.5)

        def warm_fill(uname, iters=70):
            """PE-only junk matmuls to keep HAM awake across an AG wait."""
            jp = ps_mm.tile([P, 512], f32, name=f"jp_{uname}", tag="mm")
            for i in range(iters):
                nc.tensor.matmul(jp[:], ident[:], junk16[:],
                                 start=(i == 0), stop=(i == iters - 1))

        def layer_norm(j, out16, uname):
            st = stp.tile([P, 2, 6], f32, name=f"st_{uname}", tag="st")
            for half in range(2):
                nc.vector.bn_stats(st[:, half, :], x_sb[:, j, ts(half, 512)])
            mv = stp.tile([P, 2], f32, name=f"mv_{uname}", tag="mv")
            nc.vector.bn_aggr(mv[:], st[:])
            sd = stp.tile([P, 1], f32, name=f"sd_{uname}", tag="sd")
            nc.scalar.activation(sd[:], mv[:, 1:2], AF.Sqrt, bias=eps_col[:])
            rs = stp.tile([P, 1], f32, name=f"rs_{uname}", tag="rs")
            nc.vector.reciprocal(rs[:], sd[:])
            nc.vector.tensor_scalar(
                out16[:], x_sb[:, j, :], mv[:, 0:1], rs[:],
                ALU.subtract, ALU.mult)

        def transpose_to(hT, h, uname):
            """h [P,2,E] fp16 token-major -> hT [P, EC, 2P] fp16 feature-major."""
            for j in range(2):
                for c in range(EC):
                    pt = ps_mm.tile([P, P], f16, name=f"pt_{uname}_{j}_{c}",
                                    tag="mm")
                    nc.tensor.transpose(pt[:], h[:, j, ts(c, P)], ident[:])
                    nc.scalar.copy(hT[:, c, ts(j, P)], pt[:])

        def opt2_matmul(out16, wsb, rhsT, uname, bias=None, relu=False):
            """out16 [P, n_mt, 2P] fm <- W.T @ rhsT; wsb [P, EC, n_mt*P]."""
            n_mt = wsb.shape[2] // P
            for mt in range(n_mt):
                pm = ps_mm.tile([P, 512], f32, name=f"pm_{uname}_{mt}", tag="mm")
                for ko in range(EC):
                    nc.tensor.matmul(
                        pm[:, :2 * P], wsb[:, ko, ts(mt, P)], rhsT[:, ko, :],
                        start=(ko == 0), stop=(ko == EC - 1))
                if relu:
                    nc.scalar.activation(out16[:, mt, :], pm[:, :2 * P], AF.Relu,
                                         bias=bias[:, mt:mt + 1])
                elif bias is not None:
                    nc.vector.tensor_scalar(
                        out16[:, mt, :], pm[:, :2 * P], bias[:, mt:mt + 1], None,
                        ALU.add)
                else:
                    nc.scalar.copy(out16[:, mt, :], pm[:, :2 * P])

        def wload(src_ap, uname):
            w = wsp.tile([P, EC, src_ap.shape[1]], f16, name=f"w_{uname}",
                         tag="wst")
            nc.scalar.dma_start(w[:], src_ap.rearrange("(ko p) m -> p ko m", p=P))
            return w

        # ================================================================ layers
        for l in range(NL):
            wq = wload(Wq_d[l], f"q{l}")
            wk = wload(Wk_d[l], f"k{l}")
            wv = wload(Wv_d[l], f"v{l}")
            qkb = rowp.tile([P, 2, EC], f32, name=f"qkb_{l}", tag="qkb")
            nc.sync.dma_start(qkb[:], qkb_d[l].rearrange("q (mt p) -> p q mt", p=P))
            vrow = rowp.tile([1, E], f16, name=f"vrow_{l}", tag="vrow")
            nc.sync.dma_start(vrow[:], vrow_d[l, None, :])

            # ---- LN1 -> h1 fp16, h1T
            h1 = h16p.tile([P, 2, E], f16, name=f"h1_{l}", tag="h16")
            for j in range(2):
                layer_norm(j, h1[:, j, :], f"l1_{l}_{j}")
            h1T = hT16p.tile([P, EC, 2 * P], f16, name=f"h1T_{l}", tag="hT")
            transpose_to(h1T, h1, f"h1_{l}")

            # ---- K feature-major first -> kv_in K region, launch AG_K early
            k_fm = kvlp.tile([P, EC, 2 * P], f16, name=f"kfm_{l}", tag="kfm")
            opt2_matmul(k_fm, wk, h1T, f"k{l}", bias=qkb[:, 1])
            kv_ink = dramp.tile([KSZ], f16, name=f"kvink_{l}", tag="kvink")
            nc.sync.dma_start(
                kv_ink.rearrange("(c p t) -> p c t", c=EC, p=P), k_fm[:])
            kv_outk = dramp.tile([4, KSZ], f16, name=f"kvoutk_{l}", tag="kvoutk")
            nc.gpsimd.collective_compute(
                "AllGather", ALU.bypass, replica_groups=REPLICA_GROUPS,
                ins=[kv_ink.opt()], outs=[kv_outk.opt()])

            # ---- V token-major (opt1) + ones-row bias, then AG_V
            v_tok = kvlp.tile([P, 2, E], f16, name=f"vtok_{l}", tag="vtok")
            for j in range(2):
                for nh in range(2):
                    pv = ps_mm.tile([P, 512], f32, name=f"pv_{l}_{j}_{nh}",
                                    tag="mm")
                    for ko in range(EC):
                        nc.tensor.matmul(
                            pv[:], h1T[:, ko, ts(j, P)], wv[:, ko, ts(nh, 512)],
                            start=(ko == 0), stop=False)
                    nc.tensor.matmul(pv[:], ones_row[:, :P],
                                     vrow[:, ts(nh, 512)], start=False, stop=True)
                    nc.scalar.copy(v_tok[:, j, ts(nh, 512)], pv[:])
            kv_inv = dramp.tile([VSZ], f16, name=f"kvinv_{l}", tag="kvinv")
            nc.sync.dma_start(
                kv_inv.rearrange("(j p e) -> p j e", j=2, p=P), v_tok[:])
            kv_outv = dramp.tile([4, VSZ], f16, name=f"kvoutv_{l}", tag="kvoutv")
            nc.gpsimd.collective_compute(
                "AllGather", ALU.bypass, replica_groups=REPLICA_GROUPS,
                ins=[kv_inv.opt()], outs=[kv_outv.opt()])

            # ---- Q projection overlaps the collectives
            q_fm = qfmp.tile([P, EC, 2 * P], f16, name=f"qfm_{l}", tag="qfm")
            opt2_matmul(q_fm, wq, h1T, f"q{l}", bias=qkb[:, 0])
            warm_fill(f"kw_{l}")

            # ---- load gathered K (true order) and V (ones-augmented)
            k_all = kvallp.tile([P, EC, T], f16, name=f"kall_{l}", tag="kall")
            v_aug = kvallp.tile([P, NT, H, D + 1], f16, name=f"vaug_{l}",
                                tag="vaug")
            nc.vector.memset(v_aug[:, :, :, D:D + 1], 1.0)
            for tau in range(NT):
                r, slot = tile_owner(tau)
                kblk = kv_outk[r].rearrange("(c p t) -> p c t", c=EC, p=P)
                nc.sync.dma_start(k_all[:, :, ts(tau, P)], kblk[:, :, ts(slot, P)])
            for tau in range(NT):
                r, slot = tile_owner(tau)
                vblk = kv_outv[r].rearrange(
                    "(j p h d) -> p j h d", j=2, p=P, h=H)
                nc.sync.dma_start(v_aug[:, tau, :, :D], vblk[:, slot])

            # ---- attention, pipelined per head-pair (ro=0/64 share a chunk)
            attn = attp.tile([P, 2, E], f16, name=f"attn_{l}", tag="attn")
            for pr in range(H // 2):
                co = pr
                pS = {}
                # S for both heads of the pair back-to-back: the two heads'
                # lhsT base partitions (0 / 64) land on distinct PE row
                # groups so their matmuls overlap in the array.
                for hi in range(2):
                    h = 2 * pr + hi
                    ro = hi * D
                    pS[hi] = ps_s.tile(
                        [P, NKT, P], f32, name=f"pS_{l}_{h}", tag="pS")
                    # slots 2kt/2kt+1 = (j0,j1) x kt<4 via one N=256 matmul
                    # (one LDWEIGHTS for both query tiles); slots 8+ = j1 kt>=4
                    for kt in range(N0):
                        nc.tensor.matmul(
                            pS[hi][:, 2 * kt:2 * kt + 2, :],
                            k_all[ro:ro + D, co, ts(kt, P)],
                            q_fm[ro:ro + D, co, :],
                            start=True, stop=True)
                    for kt in range(N0, N1):
                        nc.tensor.matmul(
                            pS[hi][:, N0 + kt, :],
                            k_all[ro:ro + D, co, ts(kt, P)],
                            q_fm[ro:ro + D, co, ts(1, P)],
                            start=True, stop=True)
                p16 = {}
                for hi in range(2):
                    h = 2 * pr + hi
                    p16[hi] = p16p.tile([P, NKT, P], f16, name=f"p16_{l}_{h}",
                                        tag="p16")
                    nc.scalar.activation(
                        p16[hi][:], pS[hi][:],
                        AF.Exp, scale=float(D) ** -0.5)
                    # multiplicative causal mask (exact 0/1 triangular)
                    nc.vector.tensor_tensor(
                        p16[hi][:], p16[hi][:], mask16[:], ALU.mult)
                for hi in range(2):
                    h = 2 * pr + hi
                    for j, n_kt in ((0, N0), (1, N1)):
                        pO = ps_mm.tile([P, 512], f32, name=f"pO_{l}_{h}_{j}",
                                        tag="mm")
                        for kt in range(n_kt):
                            slot = 2 * kt + j if kt < N0 else N0 + kt
                            nc.tensor.matmul(
                                pO[:, :D + 1], p16[hi][:, slot, :],
                                v_aug[:, kt, h, :],
                                start=(kt == 0), stop=(kt == n_kt - 1))
                        rz = stp.tile([P, 1], f32, name=f"rz_{l}_{h}_{j}",
                                      tag="rz")
                        nc.vector.reciprocal(rz[:], pO[:, D:D + 1])
                        nc.vector.tensor_scalar(
                            attn[:, j, h * D:(h + 1) * D], pO[:, :D],
                            rz[:], None, ALU.mult)

            # ---- attnT, Wo (opt1: lhsT=attnT chunk, rhs=Wo) + residual
            attnT = hT16p.tile([P, EC, 2 * P], f16, name=f"attnT_{l}", tag="hT")
            transpose_to(attnT, attn, f"at_{l}")
            wo = wload(Wo_d[l], f"o{l}")
            worow = rowp.tile([1, E], f16, name=f"worow_{l}", tag="vrow")
            nc.sync.dma_start(worow[:], worow_d[l, None, :])
            for j in range(2):
                for nh in range(2):
                    po = ps_mm.tile([P, 512], f32, name=f"po_{l}_{j}_{nh}",
                                    tag="mm")
                    for ko in range(EC):
                        nc.tensor.matmul(
                            po[:], attnT[:, ko, ts(j, P)], wo[:, ko, ts(nh, 512)],
                            start=(ko == 0), stop=False)
                    nc.tensor.matmul(po[:], ones_row[:, :P],
                                     worow[:, ts(nh, 512)], start=False, stop=True)
                    nc.vector.tensor_add(x_sb[:, j, ts(nh, 512)],
                                         x_sb[:, j, ts(nh, 512)], po[:])

            # ---- LN2 -> h2, h2T
            h2 = h16p.tile([P, 2, E], f16, name=f"h2_{l}", tag="h16")
            for j in range(2):
                layer_norm(j, h2[:, j, :], f"l2_{l}_{j}")
            h2T = hT16p.tile([P, EC, 2 * P], f16, name=f"h2T_{l}", tag="hT")
            transpose_to(h2T, h2, f"h2_{l}")

            # ---- fc1 (opt2, W1 streamed in 1024-col blocks) -> mid fp16
            fc1b = rowp.tile([P, 4 * EC], f32, name=f"fc1b_{l}", tag="fc1b")
            nc.sync.dma_start(fc1b[:], fc1b_d[l].rearrange("(mt p) -> p mt", p=P))
            mid = midp.tile([P, 4 * EC, 2 * P], f16, name=f"mid_{l}", tag="mid")
            for blk in range(4):
                w1b = wload(W1_d[l, :, ts(blk, 1024)], f"1_{l}_{blk}")
                for mi in range(8):
                    mt = blk * 8 + mi
                    pm = ps_mm.tile([P, 512], f32, name=f"pf_{l}_{blk}_{mi}",
                                    tag="mm")
                    for ko in range(EC):
                        nc.tensor.matmul(
                            pm[:, :2 * P], w1b[:, ko, ts(mi, P)], h2T[:, ko, :],
                            start=(ko == 0), stop=(ko == EC - 1))
                    nc.scalar.activation(mid[:, mt, :], pm[:, :2 * P], AF.Relu,
                                         bias=fc1b[:, mt:mt + 1])

            # ---- fc2 (opt1, W2 streamed in 1024-row k-groups); kg partials
            # accumulate into x_sb via DVE adds (keeps PSUM to 2 mm banks).
            w2row = rowp.tile([1, E], f16, name=f"w2row_{l}", tag="vrow")
            nc.sync.dma_start(w2row[:], w2row_d[l, None, :])
            for kg in range(4):
                w2b = wload(W2_d[l, ts(kg, 1024), :], f"2_{l}_{kg}")
                for j in range(2):
                    for nh in range(2):
                        pfc = ps_mm.tile([P, 512], f32,
                                         name=f"pfc_{l}_{kg}_{j}_{nh}", tag="mm")
                        for ko in range(EC):
                            nc.tensor.matmul(
                                pfc[:], mid[:, kg * EC + ko, ts(j, P)],
                                w2b[:, ko, ts(nh, 512)],
                                start=(ko == 0), stop=(kg < 3 and ko == EC - 1))
                        if kg == 3:
                            nc.tensor.matmul(pfc[:], ones_row[:, :P],
                                             w2row[:, ts(nh, 512)], start=False,
                                             stop=True)
                        nc.vector.tensor_add(x_sb[:, j, ts(nh, 512)],
                                             x_sb[:, j, ts(nh, 512)], pfc[:])

            if f"xout{l}" in taps:
                nc.sync.dma_start(
                    taps[f"xout{l}"].rearrange("(j p) e -> p j e", p=P), x_sb[:])

        # ================================================================ head
        hf = h16p.tile([P, 2, E], f16, name="hf", tag="h16")
        for j in range(2):
            layer_norm(j, hf[:, j, :], f"lf_{j}")
        hfT = hT16p.tile([P, EC, 2 * P], f16, name="hfT", tag="hT")
        transpose_to(hfT, hf, "hf")
        lmw = wsp.tile([P, EC, V], f16, name="lmw", tag="wst")
        nc.scalar.dma_start(lmw[:], lmW_d.rearrange("(ko p) v -> p ko v", p=P))
        lmrow = rowp.tile([1, V], f16, name="lmrow", tag="lmrow")
        nc.sync.dma_start(lmrow[:], lmrow_d[None, :])
        out_sb = attp.tile([P, 2, V], f32, name="out_sb", tag="attn")
        for j in range(2):
            for nh in range(2):
                nv = V // 2
                pl = ps_mm.tile([P, 512], f32, name=f"pl_{j}_{nh}", tag="mm")
                for ko in range(EC):
                    nc.tensor.matmul(
                        pl[:, :nv], hfT[:, ko, ts(j, P)],
                        lmw[:, ko, ts(nh, nv)], start=(ko == 0), stop=False)
                nc.tensor.matmul(pl[:, :nv], ones_row[:, :P],
                                 lmrow[:, ts(nh, nv)], start=False, stop=True)
                nc.scalar.copy(out_sb[:, j, ts(nh, nv)], pl[:, :nv])
        nc.sync.dma_start(logits_d.rearrange("(j p) v -> p j v", p=P), out_sb[:])

    nc.compile()
    return nc


# ---------------------------------------------------------------- entry
_CACHED = {}


def run(inputs, num_layers=L, debug_taps=(), trace=False):
    key = (num_layers, tuple(t[0] for t in debug_taps))
    if key not in _CACHED:
        _CACHED[key] = build(num_layers, debug_taps)
    nc = _CACHED[key]
    in_maps = prep_host(inputs)
    return run_bass_kernel_spmd(nc, in_maps, core_ids=list(range(8)), trace=trace)


LAST_EXEC_NS = None


def kernel(**inputs):
    res = run(inputs, num_layers=L, trace=False)
    return assemble_output(res.results)
